# revision 23
# baseline (speedup 1.0000x reference)
"""Multi-head attention (B=16, N=1024, E=768, H=12) on 8 TRN2 NeuronCores.

Data parallel over batch (2 per core, no collectives). Per-core fused kernel:
  - X^T loaded directly (host pre-transposes to (E, T) bf16).
  - QKV: Q/K feature-major (each 128-chunk = two heads' Q^T/K^T, bias added
    on DVE from PSUM); V token-major into a (tok, 12*65) layout with a
    constant ones column per head.
  - energy^T per head pair as two concurrent row-tiled matmuls (K=64 at row
    offsets 0/64) into one (128,1024) PSUM tile; a single Exp ACTIVATE
    (scale=1/8, no max subtraction - |logit| < 2 by construction) drains it.
  - attn@V: lhsT = [V | 1] (M=65); PSUM row 64 = softmax denominators.
  - Normalization: reciprocal_approx_fast + gpsimd partition broadcast,
    fused into the PSUM->SBUF copy that writes the shifted-duplicate
    "DOUBLE" layout; stride-12 APs over DOUBLE give exact 128-row slabs of
    Y^T for the reference's scrambled (H,N,D)->(N,E) reshape, so the out
    projection is 6 clean K=128 accumulating matmuls per 128-token tile.
  - The attention phase is ACT-bound (exp of 25.2M logits at 1 elem/lane/
    cycle): all deferred PE work (late Q/K chunks, late V chunks, early out
    projection) is spread thin through a "pump" - at most a couple of
    matmuls per energy/attn@V slot, on a dedicated 1-bank PSUM pool - so
    the energy PSUM ping-pong never blocks and the ACT engine runs
    back-to-back.
  - Startup: x (half-tiles) and weight strips ordered first-needed-first
    across the three DMA-capable queues (sync/scalar HWDGE + gpsimd SWDGE,
    ~45GB/s each) while dummy matmuls warm the PE HAM clock gate; output
    is written as bf16 (converted to fp32 on host) to halve the final
    out-DMA drain.
"""

import contextlib

import numpy as np

import concourse.bass as bass
import concourse.tile as tile
import concourse.mybir as mybir
from concourse import bacc
from concourse import bass_utils

B, N, E, H = 16, 1024, 768, 12
D = E // H          # 64
N_CORES = 8
BPC = B // N_CORES  # 2
T = BPC * N         # 2048
F3 = 3 * E
SCALE = 1.0 / float(np.sqrt(np.float32(D)))

FP32 = mybir.dt.float32
BF16 = mybir.dt.bfloat16
INT32 = mybir.dt.int32
AF = mybir.ActivationFunctionType
OP = mybir.AluOpType


def _emit(tc, x_ap, wqkv_ap, bqkv_ap, wout_ap, bout_ap, out_ap):
    nc = tc.nc
    EC = E // 128      # 6
    FC = 2 * E // 128  # 12
    TC4 = T // 512     # 4
    TC16 = T // 128    # 16
    HM = H * N         # 12288

    stack = contextlib.ExitStack()
    with stack:
        const_pool = stack.enter_context(tc.tile_pool(name="const", bufs=1))
        w_pool = stack.enter_context(tc.tile_pool(name="w", bufs=1))
        qkt_pool = stack.enter_context(tc.tile_pool(name="qkt", bufs=1))
        vo_pool = stack.enter_context(tc.tile_pool(name="vo", bufs=1))
        dbl_pool = stack.enter_context(tc.tile_pool(name="dbl", bufs=1))

        pse = stack.enter_context(
            tc.tile_pool(name="pse", bufs=2, space="PSUM"))   # (128,1024) = 2 banks
        pso = stack.enter_context(
            tc.tile_pool(name="pso", bufs=3, space="PSUM"))   # (65,512) = 1 bank
        psq = stack.enter_context(
            tc.tile_pool(name="psq", bufs=1, space="PSUM"))   # (128,512) = 1 bank

        # ---- warm tile memset first so warm-up matmuls start at t=0 -----
        warm = const_pool.tile([128, 256], BF16, tag="warm")
        nc.vector.memset(warm[:, :], 0.0)

        # ---- constants (gpsimd queue, small, first) ---------------------
        bq = const_pool.tile([128, FC], FP32, tag="bq")
        nc.gpsimd.dma_start(bq[:, :], bqkv_ap.rearrange("(c p) -> p c", p=128)[:, 0:FC])
        bv_row = const_pool.tile([1, E], BF16, tag="brow", name="bv_row")
        nc.gpsimd.dma_start(bv_row[:, :], bqkv_ap[2 * E:3 * E].unsqueeze(0))
        bo_row = const_pool.tile([1, E], BF16, tag="brow", name="bo_row")
        nc.gpsimd.dma_start(bo_row[:, :], bout_ap.unsqueeze(0))

        # ---- X load: full tiles, split over the two HWDGE queues --------
        xt_pool = stack.enter_context(tc.tile_pool(name="xt", bufs=1))
        xt = [xt_pool.tile([128, T], BF16, tag=f"xt{ec}", name=f"xt{ec}")
              for ec in range(EC)]
        wsb = [w_pool.tile([128, F3], BF16, tag=f"wsb{ec}", name=f"wsb{ec}")
               for ec in range(EC)]
        wosb = [w_pool.tile([128, E], BF16, tag=f"wosb{ec}", name=f"wosb{ec}")
                for ec in range(EC)]
        # broadcasts first on gpsimd (V chunks need bv early)
        bv = const_pool.tile([128, E], BF16, tag="bv")
        nc.gpsimd.partition_broadcast(bv[:, :], bv_row[:, :], channels=128)
        bo = const_pool.tile([128, E], BF16, tag="bo")
        nc.gpsimd.partition_broadcast(bo[:, :], bo_row[:, :], channels=128)
        # x in half-tiles, absolutely first on each queue: the first token
        # half unblocks fci0 tch0/1 + the b0 V chunks ~12us early
        xqs = [nc.sync, nc.scalar, nc.gpsimd] * 2
        for hlf in range(2):
            for ec in range(EC):
                xqs[ec].dma_start(
                    xt[ec][:, hlf * 1024:(hlf + 1) * 1024],
                    x_ap[ec * 128:(ec + 1) * 128, hlf * 1024:(hlf + 1) * 1024])
        # strips A=[Q fc0,1] B=[K fc6,7] behind x on the HWDGE queues
        for ec in range(EC):
            nc.sync.dma_start(wsb[ec][:, 0:256],
                              wqkv_ap[ec * 128:(ec + 1) * 128, 0:256])
            nc.scalar.dma_start(wsb[ec][:, 768:1024],
                                wqkv_ap[ec * 128:(ec + 1) * 128, 768:1024])
        # C=[V] on scalar (needed mid-head), D/E/wosb trail on gpsimd
        for ec in range(EC):
            nc.scalar.dma_start(wsb[ec][:, 1536:2304],
                                wqkv_ap[ec * 128:(ec + 1) * 128, 1536:2304])
        for lo, hi in [(256, 768), (1024, 1536)]:
            for ec in range(EC):
                nc.gpsimd.dma_start(wsb[ec][:, lo:hi],
                                    wqkv_ap[ec * 128:(ec + 1) * 128, lo:hi])
        for ec in range(EC):   # out-proj weights: needed last
            nc.gpsimd.dma_start(wosb[ec][:, :],
                                wout_ap[ec * 128:(ec + 1) * 128, :])

        # ---- PE warm-up: dummy matmuls during the x DMA wait ------------
        wps = psq.tile([128, 512], FP32, tag="psq", name="warmps")
        for _ in range(160):
            nc.tensor.matmul(wps[:, 0:256], warm[:, 0:128], warm[:, :],
                             start=True, stop=True)

        # ---- Q/K production (rotating 3-deep tile slots) ----------------
        qtiles, ktiles = {}, {}

        def qk_tile(fci, kind):
            tiles = qtiles if kind == 0 else ktiles
            if fci not in tiles:
                tiles[fci] = qkt_pool.tile(
                    [128, T], BF16, tag=f"{'qk'[kind]}{fci % 3}",
                    name=f"{'qk'[kind]}t{fci}")
            return tiles[fci]

        def emit_qk_psum(fci, kind, tch):
            """One full psum group on pse: Q or K chunk, head phase only."""
            tiles = qk_tile(fci, kind)
            fc = fci + 6 * kind
            ps = pse.tile([128, 1024], FP32, tag="pse")
            for ec in range(EC):
                nc.tensor.matmul(
                    ps[:, 0:512],
                    wsb[ec][:, fc * 128:(fc + 1) * 128],
                    xt[ec][:, tch * 512:(tch + 1) * 512],
                    start=(ec == 0), stop=(ec == EC - 1))
            nc.vector.tensor_scalar_add(
                tiles[:, tch * 512:(tch + 1) * 512], ps[:, 0:512],
                bq[:, fc:fc + 1])

        # ---- QKV: V path -> VO (tok-major, ones col per head) -----------
        vo = [vo_pool.tile([128, H * (D + 1)], BF16, tag=f"vo{i}",
                           name=f"vo{i}") for i in range(TC16)]

        def emit_v_chunk(tc16):
            """Full-width V chunk on pse (head phase)."""
            ps = pse.tile([128, 1024], FP32, tag="pse")
            for ec in range(EC):
                nc.tensor.matmul(
                    ps[:, 0:512],
                    xt[ec][:, tc16 * 128:(tc16 + 1) * 128],
                    wsb[ec][:, 2 * E:2 * E + 512],
                    start=(ec == 0), stop=(ec == EC - 1))
            for ec in range(EC):
                nc.tensor.matmul(
                    ps[:, 512:768],
                    xt[ec][:, tc16 * 128:(tc16 + 1) * 128],
                    wsb[ec][:, 2 * E + 512:3 * E],
                    start=(ec == 0), stop=(ec == EC - 1))
            nc.vector.memset(vo[tc16][:, D::(D + 1)], 1.0)
            vo3a = vo[tc16][:, 0:8 * (D + 1)].rearrange(
                "p (h j) -> p h j", j=D + 1)[:, :, 0:D]
            nc.vector.tensor_tensor(
                vo3a, ps[:, 0:512].rearrange("p (h j) -> p h j", j=D),
                bv[:, 0:512].rearrange("p (h j) -> p h j", j=D), op=OP.add)
            vo3b = vo[tc16][:, 8 * (D + 1):].rearrange(
                "p (h j) -> p h j", j=D + 1)[:, :, 0:D]
            nc.vector.tensor_tensor(
                vo3b, ps[:, 512:768].rearrange("p (h j) -> p h j", j=D),
                bv[:, 512:768].rearrange("p (h j) -> p h j", j=D), op=OP.add)

        # ---- pump: deferred PE work, emitted ~2 matmuls per slot --------
        # Each item is a closure emitting one matmul (plus trailing DVE
        # fixups); all deferred work goes through the dedicated psq bank so
        # the energy ping-pong (pse) is never blocked and ACT stays fed.
        pump_q = []          # list of (gate_group, chunk_key, is_first, op)
        pump_state = {}
        pump_started = set()

        def pump_qk_chunk(fci, kind, tch):
            tiles = qk_tile(fci, kind)
            fc = fci + 6 * kind
            key = f"qk{fci}_{kind}_{tch}"

            def mk(ec):
                def go():
                    if ec == 0:
                        pump_state[key] = psq.tile([128, 512], FP32, tag="psq", name=key)
                    ps = pump_state[key]
                    nc.tensor.matmul(
                        ps[:, :],
                        wsb[ec][:, fc * 128:(fc + 1) * 128],
                        xt[ec][:, tch * 512:(tch + 1) * 512],
                        start=(ec == 0), stop=(ec == EC - 1))
                    if ec == EC - 1:
                        nc.vector.tensor_scalar_add(
                            tiles[:, tch * 512:(tch + 1) * 512], ps[:, :],
                            bq[:, fc:fc + 1])
                return go
            return [mk(ec) for ec in range(EC)]

        def pump_v_chunk(tc16):
            key = f"v{tc16}"
            ops = []

            def mk(half, ec):
                lo = 2 * E + half * 512
                wid = 512 if half == 0 else 256

                def go():
                    if ec == 0 and half == 0:
                        nc.vector.memset(vo[tc16][:, D::(D + 1)], 1.0)
                    if ec == 0:
                        pump_state[key] = psq.tile([128, 512], FP32, tag="psq", name=key)
                    ps = pump_state[key]
                    nc.tensor.matmul(
                        ps[:, 0:wid],
                        xt[ec][:, tc16 * 128:(tc16 + 1) * 128],
                        wsb[ec][:, lo:lo + wid],
                        start=(ec == 0), stop=(ec == EC - 1))
                    if ec == EC - 1:
                        if half == 0:
                            vo3 = vo[tc16][:, 0:8 * (D + 1)].rearrange(
                                "p (h j) -> p h j", j=D + 1)[:, :, 0:D]
                            bvs = bv[:, 0:512]
                        else:
                            vo3 = vo[tc16][:, 8 * (D + 1):].rearrange(
                                "p (h j) -> p h j", j=D + 1)[:, :, 0:D]
                            bvs = bv[:, 512:768]
                        nc.vector.tensor_tensor(
                            vo3, ps[:, 0:wid].rearrange("p (h j) -> p h j", j=D),
                            bvs.rearrange("p (h j) -> p h j", j=D), op=OP.add)
                return go
            for half in range(2):
                for ec in range(EC):
                    ops.append(mk(half, ec))
            return ops

        osb_pool = stack.enter_context(tc.tile_pool(name="osb", bufs=2))
        dbl = [dbl_pool.tile([128, HM], BF16, tag=f"dbl{b}", name=f"dbl{b}")
               for b in range(BPC)]
        outproj_done = set()

        def pump_outproj_chunk(b, npc):
            """Out-projection chunk via psq (two passes: 512 + 256)."""
            outproj_done.add((b, npc))
            key = f"op{b}_{npc}"
            ops = []

            def mk(half, cc):
                lo = half * 512
                wid = 512 if half == 0 else 256

                def go():
                    if cc == 0 and half == 0:
                        pump_state[key + "osb"] = osb_pool.tile(
                            [128, E], BF16, tag="osb", name=key + "osb")
                    if cc == 0:
                        pump_state[key] = psq.tile([128, 512], FP32, tag="psq", name=key)
                    ps = pump_state[key]
                    off = 2 * cc + 12 * (npc * 128)
                    lhsT = dbl[b][:, off::12][:, 0:128]
                    nc.tensor.matmul(ps[:, 0:wid], lhsT, wosb[cc][:, lo:lo + wid],
                                     start=(cc == 0), stop=(cc == EC - 1))
                    if cc == EC - 1:
                        osb = pump_state[key + "osb"]
                        nc.vector.tensor_tensor(
                            osb[:, lo:lo + wid], ps[:, 0:wid],
                            bo[:, lo:lo + wid], op=OP.add)
                        if half == 1:
                            oeng = (nc.sync, nc.scalar, nc.gpsimd)[npc % 3]
                            oeng.dma_start(
                                out_ap[b * N + npc * 128:
                                       b * N + (npc + 1) * 128, :],
                                osb[:, :])
                return go
            for half in range(2):
                for cc in range(EC):
                    ops.append(mk(half, cc))
            return ops

        def pump_step(group_idx, budget=1):
            if group_idx >= 21:
                budget = 3      # qk queue is long gone; push outproj
            while budget > 0 and pump_q:
                gate, key, first, op = pump_q[0]
                if gate > group_idx:
                    return
                pump_q.pop(0)
                if first:
                    pump_started.add(key)
                op()
                budget -= 1

        # ---- head phase -------------------------------------------------
        emit_qk_psum(0, 0, 0)
        emit_qk_psum(0, 1, 0)
        emit_qk_psum(0, 0, 1)
        emit_qk_psum(0, 1, 1)
        for tc16 in range(8):
            emit_v_chunk(tc16)
        for tch in range(2, TC4):
            emit_qk_psum(0, 0, tch)
            emit_qk_psum(0, 1, tch)
        for tc16 in range(8, 12):
            emit_v_chunk(tc16)
        for tch in range(TC4):
            emit_qk_psum(1, 0, tch)
            emit_qk_psum(1, 1, tch)

        for tc16 in range(12, 16):
            emit_v_chunk(tc16)
        for tch in range(TC4):
            emit_qk_psum(2, 0, tch)
            emit_qk_psum(2, 1, tch)

        # deferred work into the pump (order matters; gates only for dbl).
        # fci=3 needed at group 12 (slot 96), fci=4 at 16 (128), fci=5 at
        # 20 (160); at budget=1/slot they emit by slots 48/96/144.
        for fci in range(3, 6):
            for kind in range(2):
                for tch in range(TC4):
                    ops = pump_qk_chunk(fci, kind, tch)
                    for j, op in enumerate(ops):
                        pump_q.append((-1, None, False, op))
        # early out-projection for batch 0 (dbl[0] complete after the
        # (fci=5, b=0, tq=1) drain, which lands during group 22)
        for npc in range(N // 128):
            gate = 22 if npc < 4 else 23
            ops = pump_outproj_chunk(0, npc)
            for j, op in enumerate(ops):
                pump_q.append((gate, (0, npc), j == 0, op))

        # ---- attention + out projection, software pipelined -------------
        et_pool = stack.enter_context(tc.tile_pool(name="et", bufs=14))
        small_pool = stack.enter_context(tc.tile_pool(name="small", bufs=1))
        rb_pool = stack.enter_context(tc.tile_pool(name="rb", bufs=2))
        stage_pool = stack.enter_context(tc.tile_pool(name="stage", bufs=2))

        def alloc_pos():
            return [pso.tile([65, 512], FP32, tag="po", name=f"po{h}")
                    for h in range(2)]

        def emit_attnv_tk(st, pos, tk):
            """attn@V matmuls (both halves) for one tk chunk of sub-group st."""
            b, fc, tq, ets = st
            for half in range(2):
                h = 2 * fc + half
                nc.tensor.matmul(
                    pos[half][:, :],
                    vo[b * 8 + tk][:, h * (D + 1):(h + 1) * (D + 1)],
                    ets[tk][:, half * 512:(half + 1) * 512],
                    start=(tk == 0), stop=(tk == 7))

        def emit_drain(st, pos):
            b, fc, tq, _ = st
            for half in range(2):
                h = 2 * fc + half
                po = pos[half]
                sraw = small_pool.tile([1, 512], FP32, tag="sraw")
                nc.vector.tensor_copy(sraw[:, :], po[D:D + 1, :])
                rec = small_pool.tile([1, 512], FP32, tag="rec")
                nc.vector.reciprocal_approx_fast(rec[:, :], sraw[:, :])
                rb = rb_pool.tile([64, 512], FP32, tag="rb")
                nc.gpsimd.partition_broadcast(rb[:, :], rec[:, :], channels=64)
                m0 = h * N + tq * 512
                nc.vector.tensor_tensor(
                    dbl[b][0:D, m0:m0 + 512], po[0:D, :], rb[:, :], op=OP.mult)
                if m0 == 0:
                    nc.vector.tensor_tensor(
                        dbl[b][D:128, 0:511], po[0:D, 1:512], rb[:, 1:512],
                        op=OP.mult)
                else:
                    nc.vector.tensor_tensor(
                        dbl[b][D:128, m0 - 1:m0 + 511], po[0:D, :], rb[:, :],
                        op=OP.mult)

        def emit_outproj_chunk(b, npc):
            """Tail out-projection chunk on pse (full width, fast)."""
            outproj_done.add((b, npc))
            pf = pse.tile([128, 1024], FP32, tag="pse")
            for cc in range(EC):
                off = 2 * cc + 12 * (npc * 128)
                lhsT = dbl[b][:, off::12][:, 0:128]
                nc.tensor.matmul(pf[:, 0:512], lhsT, wosb[cc][:, 0:512],
                                 start=(cc == 0), stop=(cc == EC - 1))
            for cc in range(EC):
                off = 2 * cc + 12 * (npc * 128)
                lhsT = dbl[b][:, off::12][:, 0:128]
                nc.tensor.matmul(pf[:, 512:768], lhsT, wosb[cc][:, 512:768],
                                 start=(cc == 0), stop=(cc == EC - 1))
            osb = osb_pool.tile([128, E], BF16, tag="osb")
            nc.vector.tensor_tensor(osb[:, :], pf[:, 0:768], bo[:, :], op=OP.add)
            oeng = (nc.sync, nc.scalar, nc.gpsimd)[npc % 3]
            oeng.dma_start(
                out_ap[b * N + npc * 128:b * N + (npc + 1) * 128, :], osb[:, :])

        prev = None           # (b, fc, tq, ets) awaiting attn@V
        group_idx = 0
        for fci in range(H // 2):
            for b in range(BPC):
                for tq in range(2):
                    ets = []
                    prev_pos = alloc_pos() if prev is not None else None
                    for tk in range(8):
                        pe = pse.tile([128, 1024], FP32, tag="pse")
                        for half in range(2):
                            lo = 64 * half
                            nc.tensor.matmul(
                                pe[:, half * 512:(half + 1) * 512],
                                ktiles[fci][lo:lo + 64,
                                            b * N + tk * 128:b * N + (tk + 1) * 128],
                                qtiles[fci][lo:lo + 64,
                                            b * N + tq * 512:b * N + (tq + 1) * 512],
                                start=True, stop=True)
                        et = et_pool.tile([128, 1024], BF16, tag="et")
                        nc.scalar.activation(et[:, :], pe[:, :], AF.Exp,
                                             bias=0.0, scale=SCALE)
                        ets.append(et)
                        if prev is not None:
                            emit_attnv_tk(prev, prev_pos, tk)
                        pump_step(group_idx, budget=1)
                    if prev is not None:
                        emit_drain(prev, prev_pos)
                    prev = (b, fci, tq, ets)
                    group_idx += 1
        prev_pos = alloc_pos()
        for tk in range(8):
            emit_attnv_tk(prev, prev_pos, tk)
            pump_step(group_idx, budget=1)
        emit_drain(prev, prev_pos)
        # flush the pump: finish chunks already in flight; drop chunks not
        # yet started (the pse-based tail path is much faster than psq)
        for gate, key, first, op in pump_q:
            if key is not None and key not in pump_started:
                if first:
                    outproj_done.discard(key)
                continue
            op()
        pump_q.clear()
        # tail: whatever out-projection chunks remain, full-rate on pse
        for b in range(BPC):
            for npc in range(N // 128):
                if (b, npc) not in outproj_done:
                    emit_outproj_chunk(b, npc)


_built = None


def _build():
    global _built
    if _built is not None:
        return _built
    nc = bacc.Bacc("TRN2", target_bir_lowering=False, debug=False,
                   num_devices=N_CORES)
    x_ap = nc.dram_tensor("x", (E, T), BF16, kind="ExternalInput").ap()
    wqkv_ap = nc.dram_tensor("w_qkv", (E, F3), BF16, kind="ExternalInput").ap()
    bqkv_ap = nc.dram_tensor("b_qkv", (F3,), FP32, kind="ExternalInput").ap()
    wout_ap = nc.dram_tensor("w_out", (E, E), BF16, kind="ExternalInput").ap()
    bout_ap = nc.dram_tensor("b_out", (E,), FP32, kind="ExternalInput").ap()
    out_ap = nc.dram_tensor("out", (T, E), BF16, kind="ExternalOutput").ap()
    with tile.TileContext(nc) as tc:
        _emit(tc, x_ap, wqkv_ap, bqkv_ap, wout_ap, bout_ap, out_ap)
    nc.compile()
    _built = nc
    return nc


def kernel(x, W_qkv, b_qkv, W_out, b_out, _trace=False):
    import ml_dtypes
    x = np.asarray(x, dtype=np.float32).astype(ml_dtypes.bfloat16)
    xT = [np.ascontiguousarray(
        x[c * BPC:(c + 1) * BPC].reshape(T, E).T) for c in range(N_CORES)]
    W_qkv = np.ascontiguousarray(
        np.asarray(W_qkv, dtype=np.float32).astype(ml_dtypes.bfloat16))
    b_qkv = np.ascontiguousarray(np.asarray(b_qkv, dtype=np.float32))
    W_out = np.ascontiguousarray(
        np.asarray(W_out, dtype=np.float32).astype(ml_dtypes.bfloat16))
    b_out = np.ascontiguousarray(np.asarray(b_out, dtype=np.float32))

    nc = _build()
    in_maps = [
        {
            "x": xT[c],
            "w_qkv": W_qkv, "b_qkv": b_qkv, "w_out": W_out, "b_out": b_out,
        }
        for c in range(N_CORES)
    ]
    res = bass_utils.run_bass_kernel_spmd(
        nc, in_maps, core_ids=list(range(N_CORES)), trace=_trace)
    out = np.concatenate(
        [np.asarray(res.results[c]["out"], dtype=np.float32).reshape(BPC, N, E)
         for c in range(N_CORES)],
        axis=0)
    if _trace:
        kernel._last_results = res
    return out
